# revision 1
# baseline (speedup 1.0000x reference)
"""Trainium2 Bass kernel for nn_AttnProcessor (SDXL-style cross-attention with
region-prompt bias coupled through a global score max).

Sharding: data-parallel over batch -- core b handles batch element b (B=8 on 8
cores).  The global max of the attention scores couples the cores, resolved
with an on-device AllReduce(max) of one scalar.

Per-core math (hs [S,D], ehs [L,C], region [S,L]):
  qT[d,s]   = (Wq.T @ hs.T) * SCALE          (fp16 matmuls, fp32 accum)
  kT[d,l]   = Wk.T @ ehs.T ;  v[l,d] = ehs @ Wv
  scT[l,s]  = kT_h.T @ qT_h                  (per head, PSUM fp32)
  gmax      = AllReduce-max over all scT
  sc'       = scT + region.T * (log1p(.1*sigma)*gmax)
  probs     = exp(sc' - C) / sum_l exp(sc' - C)   (C = gmax*(1+log1p) const)
  attnT     = v_h.T @ probs ;  out = attnT.T @ Wo + bo

All matmul operands are fp16 (PSUM accumulation is fp32); elementwise math,
reductions and the bias path stay fp32.  Scores bounce through DRAM in fp16
(chunk-major layout) between the two passes.
"""
import numpy as np

import concourse.bass as bass
import concourse.mybir as mybir
import concourse.tile as tile
from concourse import bacc
from concourse.bass_utils import run_bass_kernel_spmd
from concourse.masks import make_identity

B, S, L, D, C_ENC, H = 8, 4096, 77, 1280, 2048, 20
DH = D // H            # 64
SCALE = DH ** -0.5     # 0.125
N_CORES = 8
CHUNK = 512
NCH = S // CHUNK       # 8
NJ = D // 128          # 10 hd-tiles
NCT_Q = D // 128       # 10 k-tiles for Q
NCT_KV = C_ENC // 128  # 16 k-tiles for K/V
DSLICES = [(0, 512), (512, 512), (1024, 256)]

f32, f16 = mybir.dt.float32, mybir.dt.float16
AX = mybir.AxisListType.X
AF = mybir.ActivationFunctionType
OP = mybir.AluOpType

_CACHE = {}


def build():
    nc = bacc.Bacc("TRN2", target_bir_lowering=False, debug=False,
                   num_devices=N_CORES)
    hs_d = nc.dram_tensor("hidden_states", [S, D], f32, kind="ExternalInput")
    ehs_d = nc.dram_tensor("encoder_hidden_states", [L, C_ENC], f32, kind="ExternalInput")
    reg_d = nc.dram_tensor("region_state", [S, L], f32, kind="ExternalInput")
    wq_d = nc.dram_tensor("Wq", [D, D], f32, kind="ExternalInput")
    wk_d = nc.dram_tensor("Wk", [C_ENC, D], f32, kind="ExternalInput")
    wv_d = nc.dram_tensor("Wv", [C_ENC, D], f32, kind="ExternalInput")
    wo_d = nc.dram_tensor("Wo", [D, D], f32, kind="ExternalInput")
    bo_d = nc.dram_tensor("bo", [D], f32, kind="ExternalInput")
    sig_d = nc.dram_tensor("sigma", [1], f32, kind="ExternalInput")
    out_d = nc.dram_tensor("out", [S, D], f32, kind="ExternalOutput")

    with tile.TileContext(nc) as tc, nc.allow_low_precision(reason="fp16 matmul kernel"):
        with tc.tile_pool(name="consts", bufs=1) as cpool, \
             tc.tile_pool(name="wpool", bufs=1) as wpool, \
             tc.tile_pool(name="big", bufs=1) as bigp, \
             tc.tile_pool(name="work", bufs=1) as wk, \
             tc.tile_pool(name="ps_big", bufs=4, space="PSUM") as psb, \
             tc.tile_pool(name="ps_med", bufs=2, space="PSUM") as psm, \
             tc.tile_pool(name="ps_sm", bufs=2, space="PSUM") as pss, \
             tc.tile_pool(name="dram", bufs=1, space="DRAM") as dr:

            # ---------------- constants ----------------
            id16 = cpool.tile([128, 128], f16)
            make_identity(nc, id16)
            id32 = cpool.tile([128, 128], f32)
            make_identity(nc, id32)
            ones77c = cpool.tile([77, 1], f16)
            nc.vector.memset(ones77c[:], 1.0)
            ones77sq = cpool.tile([77, 77], f16)
            nc.vector.memset(ones77sq[:], 1.0)
            ones128r = cpool.tile([1, 128], f16)
            nc.vector.memset(ones128r[:], 1.0)

            sig = cpool.tile([1, 1], f32)
            nc.sync.dma_start(out=sig[:], in_=sig_d.ap().rearrange("(o a) -> o a", o=1))
            c0 = cpool.tile([1, 1], f32)   # log1p(0.1*sigma)
            nc.scalar.activation(c0[:], sig[:], AF.Ln, bias=1.0, scale=0.1)

            # bo broadcast [128, D]
            bo16 = cpool.tile([1, D], f16)
            nc.gpsimd.dma_start(out=bo16[:], in_=bo_d.ap().rearrange("(o a) -> o a", o=1))

            # ---------------- phase 0: ehsT, kT, v ----------------
            ehs16 = wk.tile([L, C_ENC], f16)
            nc.gpsimd.dma_start(out=ehs16[:], in_=ehs_d[:])
            ehsT = wk.tile([128, NCT_KV * L], f16)   # [c-part, ct*77+l]
            for ct in range(NCT_KV):
                pt = pss.tile([128, 128], f16, tag="sm")
                nc.tensor.transpose(pt[:, 0:L], ehs16[:, ct * 128:(ct + 1) * 128],
                                    id16[0:L, 0:L])
                nc.vector.tensor_copy(ehsT[:, ct * L:(ct + 1) * L], pt[:, 0:L])

            # Wk tiles (streamed through the shared weight pool)
            wk_t = []
            for ct in range(NCT_KV):
                t = wpool.tile([128, D], f16, tag="w", bufs=18, name=f"wk{ct}")
                nc.gpsimd.dma_start(out=t[:], in_=wk_d[ct * 128:(ct + 1) * 128, :])
                wk_t.append(t)
            kT = wk.tile([128, NJ, L], f16)          # [hd-part, j, l]
            for j in range(NJ):
                pk = psm.tile([128, 512], f32, tag="med")
                for ct in range(NCT_KV):
                    nc.tensor.matmul(pk[:, 0:L],
                                     wk_t[ct][:, j * 128:(j + 1) * 128],
                                     ehsT[:, ct * L:(ct + 1) * L],
                                     start=(ct == 0), stop=(ct == NCT_KV - 1))
                nc.scalar.copy(kT[:, j, :], pk[:, 0:L])

            wv_t = []
            for ct in range(NCT_KV):
                t = wpool.tile([128, D], f16, tag="w", bufs=18, name=f"wv{ct}")
                nc.gpsimd.dma_start(out=t[:], in_=wv_d[ct * 128:(ct + 1) * 128, :])
                wv_t.append(t)
            v_sb = wk.tile([L, D], f16)              # [l, hd]
            for d0, dn in DSLICES:
                pv0 = psm.tile([128, 512], f32, tag="med")
                for ct in range(NCT_KV):
                    nc.tensor.matmul(pv0[0:L, 0:dn],
                                     ehsT[:, ct * L:(ct + 1) * L],
                                     wv_t[ct][:, d0:d0 + dn],
                                     start=(ct == 0), stop=(ct == NCT_KV - 1))
                nc.scalar.copy(v_sb[:, d0:d0 + dn], pv0[0:L, 0:dn])

            # ---------------- regionT (fp32, exact) ----------------
            regT = bigp.tile([L, S], f32)            # 16 KB/partition
            for ci in range(NCH):
                rin = wk.tile([128, 4, L], f32, tag="rin", bufs=1)
                nc.sync.dma_start(
                    out=rin[:],
                    in_=reg_d[ci * CHUNK:(ci + 1) * CHUNK, :]
                        .rearrange("(t p) l -> p t l", p=128))
                for t in range(4):
                    ptr = pss.tile([128, 128], f32, tag="sm")
                    nc.tensor.transpose(ptr[0:L, :], rin[:, t, :], id32[:])
                    nc.vector.tensor_copy(
                        regT[:, ci * CHUNK + t * 128: ci * CHUNK + (t + 1) * 128],
                        ptr[0:L, :])

            # ---------------- Wq tiles ----------------
            wq_t = []
            for ct in range(NCT_Q):
                t = wpool.tile([128, D], f16, tag="w", bufs=18, name=f"wq{ct}")
                nc.gpsimd.dma_start(out=t[:], in_=wq_d[ct * 128:(ct + 1) * 128, :])
                wq_t.append(t)

            # scores DRAM scratch, chunk-major: [l, chunk, head, s-in-chunk]
            sc_dram = dr.tile([L, NCH, H, CHUNK], f16)

            Mx = cpool.tile([128, 1], f32)
            nc.vector.memset(Mx[:], -3.0e38)

            # ---------------- phase 1: qT, scores, local max ----------------
            for ci in range(NCH):
                hs16 = wk.tile([128, 4, D], f16, tag="hs", bufs=2)
                nc.gpsimd.dma_start(
                    out=hs16[:],
                    in_=hs_d[ci * CHUNK:(ci + 1) * CHUNK, :]
                        .rearrange("(t p) c -> p t c", p=128))
                hsT = wk.tile([128, NCT_Q, CHUNK], f16, tag="hsT", bufs=1)
                for t in range(4):
                    for ct in range(NCT_Q):
                        pt = pss.tile([128, 128], f16, tag="sm")
                        nc.tensor.transpose(pt[:], hs16[:, t, ct * 128:(ct + 1) * 128],
                                            id16[:])
                        nc.vector.tensor_copy(hsT[:, ct, t * 128:(t + 1) * 128], pt[:])
                qT = wk.tile([128, NJ, CHUNK], f16, tag="qT", bufs=2)
                for j in range(NJ):
                    pq = psb.tile([128, 512], f32, tag="big")
                    for ct in range(NCT_Q):
                        nc.tensor.matmul(pq[:], wq_t[ct][:, j * 128:(j + 1) * 128],
                                         hsT[:, ct, :],
                                         start=(ct == 0), stop=(ct == NCT_Q - 1))
                    nc.scalar.activation(qT[:, j, :], pq[:], AF.Copy,
                                         bias=0.0, scale=float(SCALE))
                stg = wk.tile([L, H, CHUNK], f16, tag="stg", bufs=2)
                for h in range(H):
                    j, r = divmod(h, 2)
                    psc = psm.tile([128, 512], f32, tag="med")
                    nc.tensor.matmul(psc[0:L, :],
                                     kT[r * 64:(r + 1) * 64, j, :],
                                     qT[r * 64:(r + 1) * 64, j, :],
                                     start=True, stop=True)
                    nc.scalar.copy(stg[:, h, :], psc[0:L, :])
                mloc = wk.tile([L, 1], f32, tag="mloc", bufs=4)
                nc.vector.reduce_max(mloc[:], stg[:].rearrange("p a b -> p (a b)"),
                                     axis=AX)
                nc.vector.tensor_tensor(out=Mx[0:L, :], in0=Mx[0:L, :],
                                        in1=mloc[:], op=OP.max)
                nc.sync.dma_start(out=sc_dram[:, ci, :, :], in_=stg[:])

            # ---------------- global max + constants ----------------
            pmx = pss.tile([128, 128], f32, tag="sm")
            nc.tensor.transpose(pmx[0:1, :], Mx[:], id32[:])
            gmx = cpool.tile([1, 1], f32)
            nc.vector.reduce_max(gmx[:], pmx[0:1, :], axis=AX)

            cin = dr.tile([1, 1], f32)
            cout = dr.tile([1, 1], f32, addr_space="Shared")
            nc.sync.dma_start(out=cin[:], in_=gmx[:])
            nc.gpsimd.collective_compute(
                "AllReduce", OP.max,
                replica_groups=[list(range(N_CORES))],
                ins=[cin.opt()], outs=[cout.opt()])
            gmaxg = cpool.tile([1, 1], f32)
            nc.sync.dma_start(out=gmaxg[:], in_=cout[:])

            cb = cpool.tile([1, 1], f32)     # log1p(0.1 sigma) * gmax
            nc.vector.tensor_tensor(out=cb[:], in0=gmaxg[:], in1=c0[:], op=OP.mult)
            cc = cpool.tile([1, 1], f32)     # C = gmax + cb  (softmax shift)
            nc.vector.tensor_tensor(out=cc[:], in0=gmaxg[:], in1=cb[:], op=OP.add)
            negC = cpool.tile([1, 1], f32)
            nc.vector.tensor_scalar_mul(negC[:], cc[:], -1.0)

            cb_d = dr.tile([1, 1], f32)
            nc.sync.dma_start(out=cb_d[:], in_=cb[:])
            cb_bc = cpool.tile([L, 1], f32)
            nc.sync.dma_start(out=cb_bc[:], in_=cb_d[:].broadcast_to((L, 1)))
            negC_d = dr.tile([1, 1], f32)
            nc.sync.dma_start(out=negC_d[:], in_=negC[:])
            negC_bc = cpool.tile([L, 1], f32)
            nc.sync.dma_start(out=negC_bc[:], in_=negC_d[:].broadcast_to((L, 1)))

            # ---------------- Wo tiles (reuse weight pool slots) ----------------
            wo_t = []
            for ct in range(NJ):
                t = wpool.tile([128, D], f16, tag="w", bufs=18, name=f"wo{ct}")
                nc.gpsimd.dma_start(out=t[:], in_=wo_d[ct * 128:(ct + 1) * 128, :])
                wo_t.append(t)

            # ---------------- phase 2: softmax, PV, output ----------------
            for ci in range(NCH):
                scs = wk.tile([L, H, CHUNK], f16, tag="stg", bufs=2)
                nc.sync.dma_start(out=scs[:], in_=sc_dram[:, ci, :, :])
                rgb = wk.tile([L, CHUNK], f32, tag="rgb", bufs=2)
                nc.vector.tensor_scalar(out=rgb[:], in0=regT[:, ci * CHUNK:(ci + 1) * CHUNK],
                                        scalar1=cb_bc[:], scalar2=None, op0=OP.mult)
                # exp(bias - C), once per chunk; probs factorizes as
                # exp(s) * exp(bias - C) * (1/den)
                expb = wk.tile([L, CHUNK], f16, tag="expb", bufs=2)
                nc.scalar.activation(expb[:], rgb[:], AF.Exp,
                                     bias=negC_bc[:], scale=1.0)
                attnT = wk.tile([128, NJ, CHUNK], f16, tag="attnT", bufs=1)
                for h in range(H):
                    j, r = divmod(h, 2)
                    es = wk.tile([L, CHUNK], f16, tag="es", bufs=4)
                    nc.scalar.activation(es[:], scs[:, h, :], AF.Exp)
                    ex = wk.tile([L, CHUNK], f16, tag="exf", bufs=4)
                    nc.vector.tensor_tensor(out=ex[:], in0=es[:], in1=expb[:],
                                            op=OP.mult)
                    pdenb = psm.tile([128, 512], f32, tag="med")
                    nc.tensor.matmul(pdenb[0:L, :], ones77sq[:], ex[:],
                                     start=True, stop=True)
                    rd = wk.tile([L, CHUNK], f16, tag="rd", bufs=4)
                    nc.vector.reciprocal(rd[:], pdenb[0:L, :])
                    probs = wk.tile([L, CHUNK], f16, tag="probs", bufs=3)
                    nc.vector.tensor_tensor(out=probs[:], in0=ex[:],
                                            in1=rd[:], op=OP.mult)
                    ppv = pss.tile([128, 512], f32, tag="sm")
                    nc.tensor.matmul(ppv[r * 64:(r + 1) * 64, :],
                                     v_sb[:, h * 64:(h + 1) * 64], probs[:],
                                     start=True, stop=True)
                    nc.scalar.copy(attnT[r * 64:(r + 1) * 64, j, :],
                                   ppv[r * 64:(r + 1) * 64, :])
                out_sb = wk.tile([128, D], f32, tag="osb", bufs=2)
                for st in range(4):
                    for d0, dn in DSLICES:
                        pf = psb.tile([128, 512], f32, tag="big")
                        for j in range(NJ):
                            nc.tensor.matmul(pf[:, 0:dn],
                                             attnT[:, j, st * 128:(st + 1) * 128],
                                             wo_t[j][:, d0:d0 + dn],
                                             start=(j == 0), stop=False)
                        nc.tensor.matmul(pf[:, 0:dn], ones128r[:],
                                         bo16[:, d0:d0 + dn],
                                         start=False, stop=True)
                        nc.vector.tensor_copy(out_sb[:, d0:d0 + dn], pf[:, 0:dn])
                    nc.sync.dma_start(
                        out=out_d[ci * CHUNK + st * 128: ci * CHUNK + (st + 1) * 128, :],
                        in_=out_sb[:])
                    if st < 3:
                        out_sb = wk.tile([128, D], f32, tag="osb", bufs=2)
    nc.compile()
    return nc


def kernel(hidden_states, encoder_hidden_states, region_state,
           Wq, Wk, Wv, Wo, bo, sigma):
    if "nc" not in _CACHE:
        _CACHE["nc"] = build()
    nc = _CACHE["nc"]
    in_maps = []
    for b in range(N_CORES):
        in_maps.append({
            "hidden_states": np.ascontiguousarray(hidden_states[b], dtype=np.float32),
            "encoder_hidden_states": np.ascontiguousarray(encoder_hidden_states[b], dtype=np.float32),
            "region_state": np.ascontiguousarray(region_state[b], dtype=np.float32),
            "Wq": np.asarray(Wq, dtype=np.float32),
            "Wk": np.asarray(Wk, dtype=np.float32),
            "Wv": np.asarray(Wv, dtype=np.float32),
            "Wo": np.asarray(Wo, dtype=np.float32),
            "bo": np.asarray(bo, dtype=np.float32),
            "sigma": np.asarray(sigma, dtype=np.float32),
        })
    r = run_bass_kernel_spmd(nc, in_maps, list(range(N_CORES)))
    return np.stack([r.results[c]["out"] for c in range(N_CORES)], axis=0)



# revision 15
# speedup vs baseline: 1.2476x; 1.2476x over previous
"""Trainium2 Bass kernel for nn_AttnProcessor (SDXL-style cross-attention with
region-prompt bias coupled through a global score max).

Sharding: data-parallel over batch -- core b handles batch element b (B=8 on 8
cores).  The global max of the attention scores couples the cores, resolved
with an on-device AllReduce(max) of one scalar.

Per-core math (hs [S,D], ehs [L,C], region [S,L]):
  qT[d,s]   = Wq.T @ hs.T                       (hs.T via DMA xbar transpose)
  kT[d,l]   = (Wk.T @ ehs.T) * SCALE ;  v[l,d] = ehs @ Wv
  s[l,s]    = kT_h.T @ qT_h                     (per head, PSUM fp32)
  es        = exp(s)  (fp16, safe: |s| < 5)     -> DRAM scratch
  gmax      = ln(AllReduce-max(max es))
  expb      = exp(region.T*cb - C), cb = log1p(.1 sigma)*gmax, C = gmax+cb
  probs     = es*expb / sum_l(es*expb)
  attnT     = v_h.T @ probs ;  out = attnT.T @ Wo + bo

Matmul operands fp16 (fp32 PSUM accum).  hs/region/ehs transposes run on the
DMA xbar (out[a,b,c] = in[c,128b+a]).  Phase 1 software-pipelines Qproj(ci)
with scores/exp(ci-1); phase 2 interleaves Oproj(ci-1) psum-groups ahead of
the Pool/DVE-paced softmax heads so the PE never drains.  Elementwise work is
split across Pool (even-head ex), DVE (odd ex, recip, probs, max), ACT (exp,
psum->sbuf copies).
"""
import numpy as np

import concourse.bass as bass
import concourse.mybir as mybir
import concourse.tile as tile
from concourse import bacc
from concourse.bass_utils import run_bass_kernel_spmd
from concourse.masks import make_identity

B, S, L, D, C_ENC, H = 8, 4096, 77, 1280, 2048, 20
DH = D // H            # 64
SCALE = DH ** -0.5     # 0.125
N_CORES = 8
CHUNK = 512
NCH = S // CHUNK       # 8
NJ = D // 128          # 10 d-tiles
NCT_KV = C_ENC // 128  # 16 c-tiles for K/V
DSLICES = [(0, 512), (512, 512), (1024, 256)]
HH = H // 2            # es staging half (10 heads)

f32, f16 = mybir.dt.float32, mybir.dt.float16
AX = mybir.AxisListType.X
AF = mybir.ActivationFunctionType
OP = mybir.AluOpType

_CACHE = {}


def build():
    nc = bacc.Bacc("TRN2", target_bir_lowering=False, debug=False,
                   num_devices=N_CORES)
    hs_d = nc.dram_tensor("hidden_states", [S, D], f32, kind="ExternalInput")
    ehs_d = nc.dram_tensor("encoder_hidden_states", [L, C_ENC], f32, kind="ExternalInput")
    reg_d = nc.dram_tensor("region_state", [S, L], f32, kind="ExternalInput")
    wq_d = nc.dram_tensor("Wq", [D, D], f32, kind="ExternalInput")
    wk_d = nc.dram_tensor("Wk", [C_ENC, D], f32, kind="ExternalInput")
    wv_d = nc.dram_tensor("Wv", [C_ENC, D], f32, kind="ExternalInput")
    wo_d = nc.dram_tensor("Wo", [D, D], f32, kind="ExternalInput")
    bo_d = nc.dram_tensor("bo", [D], f32, kind="ExternalInput")
    sig_d = nc.dram_tensor("sigma", [1], f32, kind="ExternalInput")
    out_d = nc.dram_tensor("out", [S, D], f32, kind="ExternalOutput")

    with tile.TileContext(nc) as tc, nc.allow_low_precision(reason="fp16 matmul kernel"):
        with tc.tile_pool(name="consts", bufs=1) as cpool, \
             tc.tile_pool(name="wq", bufs=1) as wqp, \
             tc.tile_pool(name="work", bufs=1) as wk, \
             tc.tile_pool(name="ps_q", bufs=2, space="PSUM") as psq, \
             tc.tile_pool(name="ps_sc", bufs=2, space="PSUM") as pssc, \
             tc.tile_pool(name="ps_pv", bufs=2, space="PSUM") as pspv, \
             tc.tile_pool(name="ps_o", bufs=2, space="PSUM") as pso, \
             tc.tile_pool(name="dram", bufs=1, space="DRAM") as dr:

            # ---------------- constants ----------------
            id32 = cpool.tile([128, 128], f32)
            make_identity(nc, id32)
            ones77sq = cpool.tile([77, 77], f16)
            nc.vector.memset(ones77sq[:], 1.0)
            ones128r = cpool.tile([1, 128], f16)
            nc.vector.memset(ones128r[:], 1.0)
            ones77r = cpool.tile([1, 77], f32)
            nc.vector.memset(ones77r[:], 1.0)
            ones512r = cpool.tile([1, 512], f32)
            nc.vector.memset(ones512r[:], 1.0)

            zero1 = cpool.tile([1, 1], f32)
            nc.vector.memset(zero1[:], 0.0)

            sig = cpool.tile([1, 1], f32)
            nc.sync.dma_start(out=sig[:], in_=sig_d.ap().rearrange("(o a) -> o a", o=1))
            c0 = cpool.tile([1, 1], f32)   # log1p(0.1*sigma)
            nc.scalar.activation(c0[:], sig[:], AF.Ln, bias=1.0, scale=0.1)
            bo16 = cpool.tile([1, D], f16)
            nc.gpsimd.dma_start(out=bo16[:], in_=bo_d.ap().rearrange("(o a) -> o a", o=1))

            es_dram = dr.tile([NCH, L, H * CHUNK], f16)
            Mx = cpool.tile([128, 1], f32)
            nc.vector.memset(Mx[:], 0.0)   # es >= 0

            # ---- Wq first (phase-1 critical), then hs0/ehs/wk/hs1/wv/reg ----
            hs_tiles = {}

            def issue_hs(ci):
                t = wk.tile([128, 4 * D], f16, tag="hs", bufs=1)
                nc.gpsimd.dma_start(
                    out=t[:].rearrange("p (t c) -> p t c", t=4),
                    in_=hs_d[ci * CHUNK:(ci + 1) * CHUNK, :]
                        .rearrange("(t p) c -> p t c", p=128))
                hs_tiles[ci] = t

            issue_hs(0)
            wq_t = []
            for ct in range(NJ):
                t = wqp.tile([128, D], f16, tag="wq", bufs=NJ, name=f"wq{ct}")
                nc.gpsimd.dma_start(out=t[:], in_=wq_d[ct * 128:(ct + 1) * 128, :])
                wq_t.append(t)

            with tc.tile_pool(name="wkv", bufs=1) as kvp:
                ehs16 = kvp.tile([128, C_ENC], f16)
                nc.gpsimd.dma_start(out=ehs16[0:L, :], in_=ehs_d[:])
                ehsT = kvp.tile([128, NCT_KV, 128], f16)  # [c-in-tile, ct, l(77)]
                nc.sync.dma_start_transpose(ehsT[:], ehs16[:])
                wk_t = []
                for ct in range(NCT_KV):
                    t = kvp.tile([128, D], f16, tag="wkv", bufs=NCT_KV, name=f"wk{ct}")
                    nc.gpsimd.dma_start(out=t[:], in_=wk_d[ct * 128:(ct + 1) * 128, :])
                    wk_t.append(t)
                wv_t = []
                for ct in range(NCT_KV):
                    t = kvp.tile([128, D], f16, tag="wkv", bufs=NCT_KV, name=f"wv{ct}")
                    nc.gpsimd.dma_start(out=t[:], in_=wv_d[ct * 128:(ct + 1) * 128, :])
                    wv_t.append(t)
                reg16 = kvp.tile([128, 32, 128], f16)
                nc.gpsimd.dma_start(
                    out=reg16[:, :, 0:L],
                    in_=reg_d[:].rearrange("(t p) l -> p t l", p=128))

                kT = wk.tile([128, NJ, L], f16)
                v_sb = wk.tile([L, D], f16)
                regT = wk.tile([128, 32, 128], f16)   # [l(77), t, p]; s = t*128+p

                def emit_kt():
                    for j in range(NJ):
                        pk = psq.tile([128, 512], f32, tag="q")
                        for ct in range(NCT_KV):
                            nc.tensor.matmul(pk[:, 0:L],
                                             wk_t[ct][:, j * 128:(j + 1) * 128],
                                             ehsT[:, ct, 0:L],
                                             start=(ct == 0), stop=(ct == NCT_KV - 1))
                        nc.scalar.activation(kT[:, j, :], pk[:, 0:L], AF.Copy,
                                             bias=0.0, scale=float(SCALE))

                def emit_v():
                    for d0, dn in DSLICES:
                        pv = pssc.tile([128, 512], f32, tag="sc")
                        for ct in range(NCT_KV):
                            nc.tensor.matmul(pv[0:L, 0:dn],
                                             ehsT[:, ct, 0:L],
                                             wv_t[ct][:, d0:d0 + dn],
                                             start=(ct == 0), stop=(ct == NCT_KV - 1))
                        nc.scalar.copy(v_sb[:, d0:d0 + dn], pv[0:L, 0:dn])

                def emit_regt():
                    nc.sync.dma_start_transpose(
                        regT[:], reg16[:].rearrange("p a b -> p (a b)"))

                # ---------------- phase 1 software pipeline -----------------
                qT_tiles = {}

                def emit_hst(ci):
                    # hsT[d-in-tile, t, j, p] = hs[t*128+p, j*128+d]  (SP xbar)
                    t = wk.tile([128, 4, NJ, 128], f16, tag="hsT", bufs=2)
                    nc.sync.dma_start_transpose(
                        t[:].rearrange("p a b c -> p (a b) c"), hs_tiles.pop(ci)[:])
                    return t

                def qproj_steps(ci, hsT):
                    qT = wk.tile([128, NJ, CHUNK], f16, tag="qT", bufs=2)
                    qT_tiles[ci] = qT
                    for j in range(NJ):
                        pq = psq.tile([128, 512], f32, tag="q")
                        for ct in range(NJ):
                            nc.tensor.matmul(pq[:], wq_t[ct][:, j * 128:(j + 1) * 128],
                                             hsT[:, :, ct, :],
                                             start=(ct == 0), stop=(ct == NJ - 1))
                        nc.scalar.copy(qT[:, j, :], pq[:])
                        yield

                def scores_steps(ci):
                    qT = qT_tiles.pop(ci)
                    es_st = None
                    for j in range(NJ):
                        for h in (2 * j, 2 * j + 1):
                            r = h % 2
                            if h % HH == 0:
                                es_st = wk.tile([L, HH, CHUNK], f16, tag="es", bufs=2)
                            psc = pssc.tile([128, 512], f32, tag="sc")
                            nc.tensor.matmul(psc[0:L, :],
                                             kT[r * 64:(r + 1) * 64, j, :],
                                             qT[r * 64:(r + 1) * 64, j, :],
                                             start=True, stop=True)
                            nc.scalar.activation(es_st[:, h % HH, :], psc[0:L, :], AF.Exp)
                            if h % HH == HH - 1:
                                half = h // HH
                                mloc = wk.tile([L, 1], f32, tag="mloc", bufs=4)
                                nc.vector.reduce_max(
                                    mloc[:], es_st[:].rearrange("p a b -> p (a b)"),
                                    axis=AX)
                                nc.vector.tensor_tensor(out=Mx[0:L, :], in0=Mx[0:L, :],
                                                        in1=mloc[:], op=OP.max)
                                nc.sync.dma_start(
                                    out=es_dram[ci][:, half * HH * CHUNK:(half + 1) * HH * CHUNK],
                                    in_=es_st[:].rearrange("p a b -> p (a b)"))
                        yield

                hsT_tiles = {}
                hsT_tiles[0] = emit_hst(0)
                issue_hs(1)
                for ci in range(NCH + 1):
                    # transpose for the NEXT chunk first (hsT bufs=2: its WAR
                    # cleared last slot), then the hs load behind it
                    if ci + 1 < NCH:
                        hsT_tiles[ci + 1] = emit_hst(ci + 1)
                    if ci + 2 < NCH:
                        issue_hs(ci + 2)
                    qp = qproj_steps(ci, hsT_tiles.pop(ci)) if ci < NCH else None
                    sc = scores_steps(ci - 1) if ci > 0 else None
                    for j in range(NJ):
                        if qp is not None:
                            next(qp, None)
                        if sc is not None:
                            next(sc, None)
                    if ci == 0:
                        emit_kt()
                    elif ci == 1:
                        emit_v()
                    elif ci == 2:
                        emit_regt()

            # prefetch es halves (0,0)/(0,1): transfers overlap the collective
            es2_tiles = {}

            def fetch_es(key):
                t = wk.tile([L, HH, CHUNK], f16, tag="es", bufs=2)
                nc.gpsimd.dma_start(
                    out=t[:].rearrange("p a b -> p (a b)"),
                    in_=es_dram[key[0]][:, key[1] * HH * CHUNK:(key[1] + 1) * HH * CHUNK])
                es2_tiles[key] = t

            fetch_es((0, 0))
            fetch_es((0, 1))

            # Wo loads: casting DMAs must issue on gpsimd -- placed after the
            # es prefetches so their WAR wait (wq slots free at chunk-7 Qproj)
            # blocks nothing phase-2-critical on the Pool queue
            wo_t = []
            for ct in range(NJ):
                t = wqp.tile([128, D], f16, tag="wq", bufs=NJ, name=f"wo{ct}")
                nc.gpsimd.dma_start(out=t[:], in_=wo_d[ct * 128:(ct + 1) * 128, :])
                wo_t.append(t)

            # ---------------- global max + collective -----------------------
            pmx = pssc.tile([128, 512], f32, tag="sc")
            nc.tensor.transpose(pmx[0:1, 0:128], Mx[:], id32[:])
            mxe = cpool.tile([1, 1], f32)
            nc.vector.reduce_max(mxe[:], pmx[0:1, 0:L], axis=AX)
            gml = cpool.tile([1, 1], f32)
            nc.scalar.activation(gml[:], mxe[:], AF.Ln)   # local gmax

            cin = dr.tile([1, 1], f32)
            cout = dr.tile([1, N_CORES], f32, addr_space="Shared")
            nc.sync.dma_start(out=cin[:], in_=gml[:])
            nc.gpsimd.collective_compute(
                "AllGather", OP.bypass,
                replica_groups=[list(range(N_CORES))],
                ins=[cin.opt()], outs=[cout.opt()])
            gmax8 = cpool.tile([1, N_CORES], f32)
            nc.sync.dma_start(out=gmax8[:], in_=cout[:])
            gmax = cpool.tile([1, 1], f32)
            nc.vector.reduce_max(gmax[:], gmax8[:], axis=AX)

            cb = cpool.tile([1, 1], f32)     # log1p(0.1 sigma) * gmax
            nc.vector.tensor_tensor(out=cb[:], in0=gmax[:], in1=c0[:], op=OP.mult)
            cc = cpool.tile([1, 1], f32)     # C = gmax + cb  (softmax shift)
            nc.vector.tensor_tensor(out=cc[:], in0=gmax[:], in1=cb[:], op=OP.add)
            negC = cpool.tile([1, 1], f32)
            nc.vector.tensor_tensor(out=negC[:], in0=zero1[:], in1=cc[:],
                                    op=OP.subtract)

            # negC_bc [77,1] and cbB [77,512] via rank-1 matmuls (no DRAM bounce)
            pnb = pspv.tile([128, 512], f32, tag="pv")
            nc.tensor.matmul(pnb[0:L, 0:1], ones77r[:], negC[:], start=True, stop=True)
            negC_bc = cpool.tile([L, 1], f32)
            nc.vector.tensor_copy(negC_bc[:], pnb[0:L, 0:1])
            pcr = pspv.tile([128, 512], f32, tag="pv")
            nc.tensor.matmul(pcr[0:1, :], cb[:], ones512r[:], start=True, stop=True)
            cb_row = cpool.tile([1, 512], f32)
            nc.vector.tensor_copy(cb_row[:], pcr[0:1, :])
            pcb = pspv.tile([128, 512], f32, tag="pv")
            nc.tensor.matmul(pcb[0:L, :], ones77r[:], cb_row[:], start=True, stop=True)
            cbB = cpool.tile([L, 512], f16)
            nc.scalar.copy(cbB[:], pcb[0:L, :])

            # ---------------- phase 2 software pipeline ----------------------
            attnT_tiles = {}
            NHALF = 2 * NCH

            def oproj_steps(ci):
                attnT = attnT_tiles.pop(ci)
                out_sb = None
                for st in range(4):
                    for d0, dn in DSLICES:
                        if d0 == 0:
                            out_sb = wk.tile([128, D], f32, tag="osb", bufs=2)
                        pf = pso.tile([128, 512], f32, tag="o")
                        nc.tensor.matmul(pf[:, 0:dn], ones128r[:],
                                         bo16[:, d0:d0 + dn],
                                         start=True, stop=False)
                        for j in range(NJ):
                            nc.tensor.matmul(pf[:, 0:dn],
                                             attnT[:, j, st * 128:(st + 1) * 128],
                                             wo_t[j][:, d0:d0 + dn],
                                             start=False, stop=(j == NJ - 1))
                        nc.scalar.copy(out_sb[:, d0:d0 + dn], pf[:, 0:dn])
                        if d0 + dn == D:
                            nc.sync.dma_start(
                                out=out_d[ci * CHUNK + st * 128: ci * CHUNK + (st + 1) * 128, :],
                                in_=out_sb[:])
                        yield

            def head_steps(ci):
                rgb = wk.tile([L, CHUNK], f16, tag="rgb", bufs=2)
                nc.vector.tensor_tensor(
                    out=rgb[:],
                    in0=regT[0:L, :, :].rearrange("p a b -> p (a b)")[:, ci * CHUNK:(ci + 1) * CHUNK],
                    in1=cbB[:], op=OP.mult)
                expb = wk.tile([L, CHUNK], f16, tag="expb", bufs=2)
                nc.scalar.activation(expb[:], rgb[:], AF.Exp,
                                     bias=negC_bc[:], scale=1.0)
                attnT = wk.tile([128, NJ, CHUNK], f16, tag="qT", bufs=2)
                attnT_tiles[ci] = attnT
                es2 = None
                for j in range(NJ):
                    ppv = pspv.tile([128, 512], f32, tag="pv")
                    for h in (2 * j, 2 * j + 1):
                        r = h % 2
                        if h % HH == 0:
                            es2 = es2_tiles.pop((ci, h // HH))
                        ex = wk.tile([L, CHUNK], f16, tag="ex", bufs=4)
                        eng = nc.gpsimd if r == 0 else nc.vector
                        eng.tensor_tensor(out=ex[:], in0=es2[:, h % HH, :],
                                          in1=expb[:], op=OP.mult)
                        pden = pssc.tile([128, 512], f32, tag="sc")
                        nc.tensor.matmul(pden[0:L, :], ones77sq[:], ex[:],
                                         start=True, stop=True)
                        rd = wk.tile([L, CHUNK], f16, tag="rd", bufs=4)
                        nc.vector.reciprocal(rd[:], pden[0:L, :])
                        probs = wk.tile([L, CHUNK], f16, tag="probs", bufs=4)
                        nc.vector.tensor_tensor(out=probs[:], in0=ex[:],
                                                in1=rd[:], op=OP.mult)
                        nc.tensor.matmul(ppv[r * 64:(r + 1) * 64, :],
                                         v_sb[:, h * 64:(h + 1) * 64], probs[:],
                                         start=True, stop=True)
                        if r == 1:
                            nc.scalar.copy(attnT[:, j, :], ppv[:])
                        if h % HH == HH - 1:
                            nxt = 2 * ci + h // HH + 2
                            if nxt < NHALF:
                                fetch_es((nxt // 2, nxt % 2))
                    yield

            for ci in range(NCH + 1):
                hd = head_steps(ci) if ci < NCH else None
                og = oproj_steps(ci - 1) if ci > 0 else None
                for j in range(NJ):
                    if og is not None:
                        next(og, None)
                        if j < 2:
                            next(og, None)   # 12 groups over 10 slots
                    if hd is not None:
                        next(hd, None)
                if og is not None and ci == NCH:
                    for _ in og:
                        pass
    nc.compile()
    return nc


def kernel(hidden_states, encoder_hidden_states, region_state,
           Wq, Wk, Wv, Wo, bo, sigma):
    if "nc" not in _CACHE:
        _CACHE["nc"] = build()
    nc = _CACHE["nc"]
    in_maps = []
    for b in range(N_CORES):
        in_maps.append({
            "hidden_states": np.ascontiguousarray(hidden_states[b], dtype=np.float32),
            "encoder_hidden_states": np.ascontiguousarray(encoder_hidden_states[b], dtype=np.float32),
            "region_state": np.ascontiguousarray(region_state[b], dtype=np.float32),
            "Wq": np.asarray(Wq, dtype=np.float32),
            "Wk": np.asarray(Wk, dtype=np.float32),
            "Wv": np.asarray(Wv, dtype=np.float32),
            "Wo": np.asarray(Wo, dtype=np.float32),
            "bo": np.asarray(bo, dtype=np.float32),
            "sigma": np.asarray(sigma, dtype=np.float32),
        })
    r = run_bass_kernel_spmd(nc, in_maps, list(range(N_CORES)))
    return np.stack([r.results[c]["out"] for c in range(N_CORES)], axis=0)


# revision 16
# speedup vs baseline: 2.0961x; 1.6801x over previous
"""Trainium2 Bass kernel for nn_AttnProcessor (SDXL-style cross-attention with
region-prompt bias coupled through a global score max).

Sharding: data-parallel over batch -- core b handles batch element b (B=8 on 8
cores).  The global max of the attention scores couples the cores, resolved
with an on-device AllReduce(max) of one scalar.

Per-core math (hs [S,D], ehs [L,C], region [S,L]):
  qT[d,s]   = Wq.T @ hs.T                       (hs.T via DMA xbar transpose)
  kT[d,l]   = (Wk.T @ ehs.T) * SCALE ;  v[l,d] = ehs @ Wv
  s[l,s]    = kT_h.T @ qT_h                     (per head, PSUM fp32)
  es        = exp(s)  (fp16, safe: |s| < 5)     -> DRAM scratch
  gmax      = ln(AllReduce-max(max es))
  expb      = exp(region.T*cb - C), cb = log1p(.1 sigma)*gmax, C = gmax+cb
  probs     = es*expb / sum_l(es*expb)
  attnT     = v_h.T @ probs ;  out = attnT.T @ Wo + bo

Matmul operands fp16 (fp32 PSUM accum).  hs/region/ehs transposes run on the
DMA xbar (out[a,b,c] = in[c,128b+a]).  Phase 1 software-pipelines Qproj(ci)
with scores/exp(ci-1); phase 2 interleaves Oproj(ci-1) psum-groups ahead of
the Pool/DVE-paced softmax heads so the PE never drains.  Elementwise work is
split across Pool (even-head ex), DVE (odd ex, recip, probs, max), ACT (exp,
psum->sbuf copies).
"""
import numpy as np

import concourse.bass as bass
import concourse.mybir as mybir
import concourse.tile as tile
from concourse import bacc
from concourse.bass_utils import run_bass_kernel_spmd
from concourse.masks import make_identity

B, S, L, D, C_ENC, H = 8, 4096, 77, 1280, 2048, 20
DH = D // H            # 64
SCALE = DH ** -0.5     # 0.125
N_CORES = 8
CHUNK = 512
NCH = S // CHUNK       # 8
NJ = D // 128          # 10 d-tiles
NCT_KV = C_ENC // 128  # 16 c-tiles for K/V
DSLICES = [(0, 512), (512, 512), (1024, 256)]
HH = H // 2            # es staging half (10 heads)

f32, f16 = mybir.dt.float32, mybir.dt.float16
AX = mybir.AxisListType.X
AF = mybir.ActivationFunctionType
OP = mybir.AluOpType

_CACHE = {}


def build():
    nc = bacc.Bacc("TRN2", target_bir_lowering=False, debug=False,
                   num_devices=N_CORES)
    hs_d = nc.dram_tensor("hidden_states", [S, D], f32, kind="ExternalInput")
    ehs_d = nc.dram_tensor("encoder_hidden_states", [L, C_ENC], f32, kind="ExternalInput")
    reg_d = nc.dram_tensor("region_state", [S, L], f32, kind="ExternalInput")
    wq_d = nc.dram_tensor("Wq", [D, D], f32, kind="ExternalInput")
    wk_d = nc.dram_tensor("Wk", [C_ENC, D], f32, kind="ExternalInput")
    wv_d = nc.dram_tensor("Wv", [C_ENC, D], f32, kind="ExternalInput")
    wo_d = nc.dram_tensor("Wo", [D, D], f32, kind="ExternalInput")
    bo_d = nc.dram_tensor("bo", [D], f32, kind="ExternalInput")
    sig_d = nc.dram_tensor("sigma", [1], f32, kind="ExternalInput")
    out_d = nc.dram_tensor("out", [S, D], f32, kind="ExternalOutput")

    with tile.TileContext(nc) as tc, nc.allow_low_precision(reason="fp16 matmul kernel"):
        with tc.tile_pool(name="consts", bufs=1) as cpool, \
             tc.tile_pool(name="wq", bufs=1) as wqp, \
             tc.tile_pool(name="work", bufs=1) as wk, \
             tc.tile_pool(name="ps_q", bufs=2, space="PSUM") as psq, \
             tc.tile_pool(name="ps_sc", bufs=2, space="PSUM") as pssc, \
             tc.tile_pool(name="ps_pv", bufs=2, space="PSUM") as pspv, \
             tc.tile_pool(name="ps_o", bufs=2, space="PSUM") as pso, \
             tc.tile_pool(name="dram", bufs=1, space="DRAM") as dr:

            # ---------------- constants ----------------
            id32 = cpool.tile([128, 128], f32)
            make_identity(nc, id32)
            ones77sq = cpool.tile([77, 77], f16)
            nc.vector.memset(ones77sq[:], 1.0)
            ones128r = cpool.tile([1, 128], f16)
            nc.vector.memset(ones128r[:], 1.0)
            ones77r = cpool.tile([1, 77], f32)
            nc.vector.memset(ones77r[:], 1.0)
            ones512r = cpool.tile([1, 512], f32)
            nc.vector.memset(ones512r[:], 1.0)

            zero1 = cpool.tile([1, 1], f32)
            nc.vector.memset(zero1[:], 0.0)

            sig = cpool.tile([1, 1], f32)
            nc.sync.dma_start(out=sig[:], in_=sig_d.ap().rearrange("(o a) -> o a", o=1))
            c0 = cpool.tile([1, 1], f32)   # log1p(0.1*sigma)
            nc.scalar.activation(c0[:], sig[:], AF.Ln, bias=1.0, scale=0.1)
            bo16 = cpool.tile([1, D], f16)
            nc.gpsimd.dma_start(out=bo16[:], in_=bo_d.ap().rearrange("(o a) -> o a", o=1))

            es_dram = dr.tile([NCH, L, H * CHUNK], f16)
            Mx = cpool.tile([128, 1], f32)
            nc.vector.memset(Mx[:], 0.0)   # es >= 0

            # ---- Wq first (phase-1 critical), then hs0/ehs/wk/hs1/wv/reg ----
            hs_tiles = {}

            def issue_hs(ci):
                t = wk.tile([128, 4 * D], f16, tag="hs", bufs=1)
                nc.gpsimd.dma_start(
                    out=t[:].rearrange("p (t c) -> p t c", t=4),
                    in_=hs_d[ci * CHUNK:(ci + 1) * CHUNK, :]
                        .rearrange("(t p) c -> p t c", p=128))
                hs_tiles[ci] = t

            issue_hs(0)
            wq_t = []
            for ct in range(NJ):
                t = wqp.tile([128, D], f16, tag="wq", bufs=NJ, name=f"wq{ct}")
                nc.gpsimd.dma_start(out=t[:], in_=wq_d[ct * 128:(ct + 1) * 128, :])
                wq_t.append(t)

            with tc.tile_pool(name="wkv", bufs=1) as kvp:
                ehs16 = kvp.tile([128, C_ENC], f16)
                nc.gpsimd.dma_start(out=ehs16[0:L, :], in_=ehs_d[:])
                ehsT = kvp.tile([128, NCT_KV, 128], f16)  # [c-in-tile, ct, l(77)]
                nc.sync.dma_start_transpose(ehsT[:], ehs16[:])
                wk_t = []
                for ct in range(NCT_KV):
                    t = kvp.tile([128, D], f16, tag="wkv", bufs=NCT_KV, name=f"wk{ct}")
                    nc.gpsimd.dma_start(out=t[:], in_=wk_d[ct * 128:(ct + 1) * 128, :])
                    wk_t.append(t)
                issue_hs(1)
                wv_t = []
                for ct in range(NCT_KV):
                    t = kvp.tile([128, D], f16, tag="wkv", bufs=NCT_KV, name=f"wv{ct}")
                    nc.gpsimd.dma_start(out=t[:], in_=wv_d[ct * 128:(ct + 1) * 128, :])
                    wv_t.append(t)
                reg16 = kvp.tile([128, 32, 128], f16)
                nc.gpsimd.dma_start(
                    out=reg16[:, :, 0:L],
                    in_=reg_d[:].rearrange("(t p) l -> p t l", p=128))

                kT = wk.tile([128, NJ, L], f16)
                v_sb = wk.tile([L, D], f16)
                regT = wk.tile([128, 32, 128], f16)   # [l(77), t, p]; s = t*128+p

                def emit_kt():
                    for j in range(NJ):
                        pk = psq.tile([128, 512], f32, tag="q")
                        for ct in range(NCT_KV):
                            nc.tensor.matmul(pk[:, 0:L],
                                             wk_t[ct][:, j * 128:(j + 1) * 128],
                                             ehsT[:, ct, 0:L],
                                             start=(ct == 0), stop=(ct == NCT_KV - 1))
                        nc.scalar.activation(kT[:, j, :], pk[:, 0:L], AF.Copy,
                                             bias=0.0, scale=float(SCALE))

                def emit_v():
                    for d0, dn in DSLICES:
                        pv = pssc.tile([128, 512], f32, tag="sc")
                        for ct in range(NCT_KV):
                            nc.tensor.matmul(pv[0:L, 0:dn],
                                             ehsT[:, ct, 0:L],
                                             wv_t[ct][:, d0:d0 + dn],
                                             start=(ct == 0), stop=(ct == NCT_KV - 1))
                        nc.scalar.copy(v_sb[:, d0:d0 + dn], pv[0:L, 0:dn])

                def emit_regt():
                    nc.sync.dma_start_transpose(
                        regT[:], reg16[:].rearrange("p a b -> p (a b)"))

                # ---------------- phase 1 software pipeline -----------------
                qT_tiles = {}

                def emit_hst(ci):
                    # hsT[d-in-tile, t, j, p] = hs[t*128+p, j*128+d]  (SP xbar)
                    t = wk.tile([128, 4, NJ, 128], f16, tag="hsT", bufs=2)
                    nc.sync.dma_start_transpose(
                        t[:].rearrange("p a b c -> p (a b) c"), hs_tiles.pop(ci)[:])
                    return t

                def qproj_steps(ci, hsT):
                    qT = wk.tile([128, NJ, CHUNK], f16, tag="qT", bufs=2)
                    qT_tiles[ci] = qT
                    for j in range(NJ):
                        pq = psq.tile([128, 512], f32, tag="q")
                        for ct in range(NJ):
                            nc.tensor.matmul(pq[:], wq_t[ct][:, j * 128:(j + 1) * 128],
                                             hsT[:, :, ct, :],
                                             start=(ct == 0), stop=(ct == NJ - 1))
                        nc.scalar.copy(qT[:, j, :], pq[:])
                        yield

                def scores_steps(ci):
                    qT = qT_tiles.pop(ci)
                    es_st = None
                    for j in range(NJ):
                        for h in (2 * j, 2 * j + 1):
                            r = h % 2
                            if h % HH == 0:
                                es_st = wk.tile([L, HH, CHUNK], f16, tag="es", bufs=2)
                            psc = pssc.tile([128, 512], f32, tag="sc")
                            nc.tensor.matmul(psc[0:L, :],
                                             kT[r * 64:(r + 1) * 64, j, :],
                                             qT[r * 64:(r + 1) * 64, j, :],
                                             start=True, stop=True)
                            nc.scalar.activation(es_st[:, h % HH, :], psc[0:L, :], AF.Exp)
                            if h % HH == HH - 1:
                                half = h // HH
                                mloc = wk.tile([L, 1], f32, tag="mloc", bufs=4)
                                nc.vector.reduce_max(
                                    mloc[:], es_st[:].rearrange("p a b -> p (a b)"),
                                    axis=AX)
                                nc.vector.tensor_tensor(out=Mx[0:L, :], in0=Mx[0:L, :],
                                                        in1=mloc[:], op=OP.max)
                                nc.sync.dma_start(
                                    out=es_dram[ci][:, half * HH * CHUNK:(half + 1) * HH * CHUNK],
                                    in_=es_st[:].rearrange("p a b -> p (a b)"))
                        yield

                hsT_tiles = {}
                hsT_tiles[0] = emit_hst(0)
                for ci in range(NCH + 1):
                    # transpose for the NEXT chunk first (hsT bufs=2: its WAR
                    # cleared last slot), then the hs load behind it
                    if ci + 1 < NCH:
                        hsT_tiles[ci + 1] = emit_hst(ci + 1)
                    if ci + 2 < NCH:
                        issue_hs(ci + 2)
                    qp = qproj_steps(ci, hsT_tiles.pop(ci)) if ci < NCH else None
                    sc = scores_steps(ci - 1) if ci > 0 else None
                    for j in range(NJ):
                        if qp is not None:
                            next(qp, None)
                        if sc is not None:
                            next(sc, None)
                    if ci == 0:
                        emit_kt()
                    elif ci == 1:
                        emit_v()
                    elif ci == 2:
                        emit_regt()

            # prefetch es halves (0,0)/(0,1): transfers overlap the collective
            es2_tiles = {}

            def fetch_es(key):
                t = wk.tile([L, HH, CHUNK], f16, tag="es", bufs=2)
                nc.gpsimd.dma_start(
                    out=t[:].rearrange("p a b -> p (a b)"),
                    in_=es_dram[key[0]][:, key[1] * HH * CHUNK:(key[1] + 1) * HH * CHUNK])
                es2_tiles[key] = t

            fetch_es((0, 0))
            fetch_es((0, 1))

            # Wo loads: casting DMAs must issue on gpsimd -- placed after the
            # es prefetches so their WAR wait (wq slots free at chunk-7 Qproj)
            # blocks nothing phase-2-critical on the Pool queue
            wo_t = []
            for ct in range(NJ):
                t = wqp.tile([128, D], f16, tag="wq", bufs=NJ, name=f"wo{ct}")
                nc.gpsimd.dma_start(out=t[:], in_=wo_d[ct * 128:(ct + 1) * 128, :])
                wo_t.append(t)

            # ---------------- global max + collective -----------------------
            pmx = pssc.tile([128, 512], f32, tag="sc")
            nc.tensor.transpose(pmx[0:1, 0:128], Mx[:], id32[:])
            mxe = cpool.tile([1, 1], f32)
            nc.vector.reduce_max(mxe[:], pmx[0:1, 0:L], axis=AX)
            gml = cpool.tile([1, 1], f32)
            nc.scalar.activation(gml[:], mxe[:], AF.Ln)   # local gmax

            cin = dr.tile([1, 1], f32)
            cout = dr.tile([1, N_CORES], f32, addr_space="Shared")
            nc.sync.dma_start(out=cin[:], in_=gml[:])
            nc.gpsimd.collective_compute(
                "AllGather", OP.bypass,
                replica_groups=[list(range(N_CORES))],
                ins=[cin.opt()], outs=[cout.opt()])
            gmax8 = cpool.tile([1, N_CORES], f32)
            nc.sync.dma_start(out=gmax8[:], in_=cout[:])
            gmax = cpool.tile([1, 1], f32)
            nc.vector.reduce_max(gmax[:], gmax8[:], axis=AX)

            cb = cpool.tile([1, 1], f32)     # log1p(0.1 sigma) * gmax
            nc.vector.tensor_tensor(out=cb[:], in0=gmax[:], in1=c0[:], op=OP.mult)
            cc = cpool.tile([1, 1], f32)     # C = gmax + cb  (softmax shift)
            nc.vector.tensor_tensor(out=cc[:], in0=gmax[:], in1=cb[:], op=OP.add)
            negC = cpool.tile([1, 1], f32)
            nc.vector.tensor_tensor(out=negC[:], in0=zero1[:], in1=cc[:],
                                    op=OP.subtract)

            # negC_bc [77,1] and cbB [77,512] via rank-1 matmuls (no DRAM bounce)
            pnb = pspv.tile([128, 512], f32, tag="pv")
            nc.tensor.matmul(pnb[0:L, 0:1], ones77r[:], negC[:], start=True, stop=True)
            negC_bc = cpool.tile([L, 1], f32)
            nc.vector.tensor_copy(negC_bc[:], pnb[0:L, 0:1])
            pcr = pspv.tile([128, 512], f32, tag="pv")
            nc.tensor.matmul(pcr[0:1, :], cb[:], ones512r[:], start=True, stop=True)
            cb_row = cpool.tile([1, 512], f32)
            nc.vector.tensor_copy(cb_row[:], pcr[0:1, :])
            pcb = pspv.tile([128, 512], f32, tag="pv")
            nc.tensor.matmul(pcb[0:L, :], ones77r[:], cb_row[:], start=True, stop=True)
            cbB = cpool.tile([L, 512], f16)
            nc.scalar.copy(cbB[:], pcb[0:L, :])

            # ---------------- phase 2 software pipeline ----------------------
            attnT_tiles = {}
            NHALF = 2 * NCH

            def oproj_steps(ci):
                attnT = attnT_tiles.pop(ci)
                out_sb = None
                for st in range(4):
                    for d0, dn in DSLICES:
                        if d0 == 0:
                            out_sb = wk.tile([128, D], f32, tag="osb", bufs=2)
                        pf = pso.tile([128, 512], f32, tag="o")
                        nc.tensor.matmul(pf[:, 0:dn], ones128r[:],
                                         bo16[:, d0:d0 + dn],
                                         start=True, stop=False)
                        for j in range(NJ):
                            nc.tensor.matmul(pf[:, 0:dn],
                                             attnT[:, j, st * 128:(st + 1) * 128],
                                             wo_t[j][:, d0:d0 + dn],
                                             start=False, stop=(j == NJ - 1))
                        nc.scalar.copy(out_sb[:, d0:d0 + dn], pf[:, 0:dn])
                        if d0 + dn == D:
                            nc.sync.dma_start(
                                out=out_d[ci * CHUNK + st * 128: ci * CHUNK + (st + 1) * 128, :],
                                in_=out_sb[:])
                        yield

            def head_steps(ci):
                rgb = wk.tile([L, CHUNK], f16, tag="rgb", bufs=2)
                nc.vector.tensor_tensor(
                    out=rgb[:],
                    in0=regT[0:L, :, :].rearrange("p a b -> p (a b)")[:, ci * CHUNK:(ci + 1) * CHUNK],
                    in1=cbB[:], op=OP.mult)
                expb = wk.tile([L, CHUNK], f16, tag="expb", bufs=2)
                nc.scalar.activation(expb[:], rgb[:], AF.Exp,
                                     bias=negC_bc[:], scale=1.0)
                attnT = wk.tile([128, NJ, CHUNK], f16, tag="qT", bufs=2)
                attnT_tiles[ci] = attnT
                es2 = None
                for j in range(NJ):
                    ppv = pspv.tile([128, 512], f32, tag="pv")
                    for h in (2 * j, 2 * j + 1):
                        r = h % 2
                        if h % HH == 0:
                            es2 = es2_tiles.pop((ci, h // HH))
                        ex = wk.tile([L, CHUNK], f16, tag="ex", bufs=4)
                        eng = nc.gpsimd if r == 0 else nc.vector
                        eng.tensor_tensor(out=ex[:], in0=es2[:, h % HH, :],
                                          in1=expb[:], op=OP.mult)
                        pden = pssc.tile([128, 512], f32, tag="sc")
                        nc.tensor.matmul(pden[0:L, :], ones77sq[:], ex[:],
                                         start=True, stop=True)
                        rd = wk.tile([L, CHUNK], f16, tag="rd", bufs=4)
                        nc.vector.reciprocal(rd[:], pden[0:L, :])
                        probs = wk.tile([L, CHUNK], f16, tag="probs", bufs=4)
                        nc.vector.tensor_tensor(out=probs[:], in0=ex[:],
                                                in1=rd[:], op=OP.mult)
                        nc.tensor.matmul(ppv[r * 64:(r + 1) * 64, :],
                                         v_sb[:, h * 64:(h + 1) * 64], probs[:],
                                         start=True, stop=True)
                        if r == 1:
                            nc.scalar.copy(attnT[:, j, :], ppv[:])
                        if h % HH == HH - 1:
                            nxt = 2 * ci + h // HH + 2
                            if nxt < NHALF:
                                fetch_es((nxt // 2, nxt % 2))
                    yield

            for ci in range(NCH + 1):
                hd = head_steps(ci) if ci < NCH else None
                og = oproj_steps(ci - 1) if ci > 0 else None
                for j in range(NJ):
                    if og is not None:
                        next(og, None)
                        if j < 2:
                            next(og, None)   # 12 groups over 10 slots
                    if hd is not None:
                        next(hd, None)
                if og is not None and ci == NCH:
                    for _ in og:
                        pass
    nc.compile()
    return nc


def kernel(hidden_states, encoder_hidden_states, region_state,
           Wq, Wk, Wv, Wo, bo, sigma):
    if "nc" not in _CACHE:
        _CACHE["nc"] = build()
    nc = _CACHE["nc"]
    in_maps = []
    for b in range(N_CORES):
        in_maps.append({
            "hidden_states": np.ascontiguousarray(hidden_states[b], dtype=np.float32),
            "encoder_hidden_states": np.ascontiguousarray(encoder_hidden_states[b], dtype=np.float32),
            "region_state": np.ascontiguousarray(region_state[b], dtype=np.float32),
            "Wq": np.asarray(Wq, dtype=np.float32),
            "Wk": np.asarray(Wk, dtype=np.float32),
            "Wv": np.asarray(Wv, dtype=np.float32),
            "Wo": np.asarray(Wo, dtype=np.float32),
            "bo": np.asarray(bo, dtype=np.float32),
            "sigma": np.asarray(sigma, dtype=np.float32),
        })
    r = run_bass_kernel_spmd(nc, in_maps, list(range(N_CORES)))
    return np.stack([r.results[c]["out"] for c in range(N_CORES)], axis=0)
